# revision 12
# baseline (speedup 1.0000x reference)
"""AlphaStock Trainium2 kernel (8 NeuronCores, SPMD).

Model: per-asset LSTM(T=64, H=128) + temporal attention pooling (HA), then
cross-asset attention (CAAN) over M=512 assets per batch element.

Sharding: the B*M=1024 sequences are split 128-per-core for the LSTM/HA
stage (launch A). The tiny CAAN stage runs as a second launch (B) sharded
by query rows (cores 0-3 -> batch 0, cores 4-7 -> batch 1), with the
gathered per-asset representations re-broadcast by the host between the
two launches.

Launch A inner loop (v2): the 128 local sequences run as TWO interleaved
streams of 64 so each stream's serial recurrence chain hides behind the
other stream's engine work. All four LSTM gates go through a SINGLE
tanh ACTIVATE per stream-step: the i/f/o rows of W_ih/W_hh are
pre-scaled by 1/2 on the host so sigmoid(x) = 0.5*tanh(x/2)+0.5 drops
out of the algebra. The cell state is kept as C2 = 2c and the hidden
output as h2 = 2h, which removes every standalone scale op:
    [t_g|t_i|t_f|t_o] = tanh(gates)               (1 ACT, PSUM src)
    [u|v] = (in+1)*[t_g|C2]  (paired STT)         (1 DVE op)
    C2'   = 0.5*v + u        (STT)                (1 DVE op)
    y     = tanh(0.5*C2')                         (1 ACT)
    h2    = (t_o+1)*y        (STT, written straight into rep)
rep is stored [h, (t, n)] so the h2 write is contiguous (no gpsimd
archive copy); w1/w2 are pre-halved and the final stock scale absorbs
the remaining factor of 2.
"""

import numpy as np

B, M, T, F, H = 2, 512, 64, 16, 128
N_CORES = 8
NPC = (B * M) // N_CORES  # sequences per core = 128
NS = NPC // 2             # sequences per stream = 64
G4 = 4 * H  # 512

_CACHE = {}


def _gate_perm():
    # torch gate order (i, f, g, o) -> kernel order (g, i, f, o)
    idx = np.arange(4 * H).reshape(4, H)
    return np.concatenate([idx[2], idx[0], idx[1], idx[3]])


def _build_launch_a():
    import concourse.bacc as bacc
    import concourse.tile as tile
    import concourse.bass as bass
    from concourse import mybir
    from contextlib import ExitStack

    f32 = mybir.dt.float32
    f16 = mybir.dt.float16
    AF = mybir.ActivationFunctionType
    ALU = mybir.AluOpType

    nc = bacc.Bacc("TRN2", target_bir_lowering=False, debug=False,
                   num_devices=N_CORES)

    xT = nc.dram_tensor("xT", [F + 1, T * NPC], f16, kind="ExternalInput").ap()
    wih = nc.dram_tensor("wih", [F + 1, G4], f16, kind="ExternalInput").ap()
    whh = nc.dram_tensor("whh", [H, G4], f16, kind="ExternalInput").ap()
    w1T = nc.dram_tensor("w1T", [H, H], f16, kind="ExternalInput").ap()
    w2T = nc.dram_tensor("w2T", [H, H], f16, kind="ExternalInput").ap()
    b12 = nc.dram_tensor("b12", [H, 1], f32, kind="ExternalInput").ap()
    waT = nc.dram_tensor("waT", [H, H], f16, kind="ExternalInput").ap()
    # CAAN stage inputs: [wqT | wkT | wvT | eye | wwT-col], [bq | bk | cst]
    wpk = nc.dram_tensor("wpk", [H, 4 * H + 1], f16, kind="ExternalInput").ap()
    bpk = nc.dram_tensor("bpk", [H, 3], f32, kind="ExternalInput").ap()
    scores = nc.dram_tensor("scores", [1, NPC], f32, kind="ExternalOutput").ap()

    with tile.TileContext(nc) as tc, ExitStack() as ctx:
        big = ctx.enter_context(tc.tile_pool(name="big", bufs=1))
        state = ctx.enter_context(tc.tile_pool(name="state", bufs=1))
        work = ctx.enter_context(tc.tile_pool(name="work", bufs=3))
        psum = ctx.enter_context(tc.tile_pool(name="psum", bufs=4, space="PSUM"))
        psumw = ctx.enter_context(tc.tile_pool(name="psumw", bufs=1, space="PSUM"))
        dram = ctx.enter_context(tc.tile_pool(name="dram", bufs=1, space="DRAM"))

        # ---- resident tensors
        xsb = big.tile([F + 1, T * NPC], f16, tag="xsb")   # x[f, (t, n)]
        rep = big.tile([H, T * NPC], f16, tag="rep")       # h2[h, (t, n)]

        wih_sb = state.tile([F + 1, G4], f16, tag="wih")
        whh_sb = state.tile([H, G4], f16, tag="whh")
        w1T_sb = state.tile([H, H], f16, tag="w1T")
        w2T_sb = state.tile([H, H], f16, tag="w2T")
        b12_sb = state.tile([H, 1], f32, tag="b12")
        waT_sb = state.tile([H, H], f16, tag="waT")

        wpk_sb = state.tile([H, 4 * H + 1], f16, tag="wpk")
        bpk_sb = state.tile([H, 3], f32, tag="bpk")

        # x chunk 0 + LSTM weights first (they gate step 0); everything the
        # tail stages need rides the gpsimd queue in parallel.
        XCH = T * NPC // 8
        nc.sync.dma_start(out=wih_sb, in_=wih)
        nc.sync.dma_start(out=xsb[:, 0:XCH], in_=xT[:, 0:XCH])
        nc.sync.dma_start(out=whh_sb, in_=whh)
        for j in range(1, 8):
            nc.sync.dma_start(out=xsb[:, j * XCH:(j + 1) * XCH],
                              in_=xT[:, j * XCH:(j + 1) * XCH])
        nc.gpsimd.dma_start(out=w1T_sb, in_=w1T)
        nc.gpsimd.dma_start(out=w2T_sb, in_=w2T)
        nc.gpsimd.dma_start(out=b12_sb, in_=b12)
        nc.gpsimd.dma_start(out=waT_sb, in_=waT)
        nc.gpsimd.dma_start(out=wpk_sb, in_=wpk)
        nc.gpsimd.dma_start(out=bpk_sb, in_=bpk)

        # Per-stream double-buffered step tile:
        # cols [t_g | t_i | t_f | t_o | C2 | u | v], each NS wide.
        W = [[state.tile([H, 7 * NS], f16, tag=f"W{s}{p}", name=f"W{s}{p}")
              for p in (0, 1)] for s in (0, 1)]
        for s in (0, 1):
            nc.vector.memset(W[s][0][:, 4 * NS:5 * NS], 0.0)  # C2_0 = 0

        def pair_ap(base, stride):
            # [p, 2, NS] view: two NS-wide blocks `stride` cols apart
            return bass.AP(tensor=base.tensor, offset=base.offset,
                           ap=[base.ap[0], [stride, 2], [1, NS]])

        ps_tiles = {}

        def emit_x(t):
            for s in (0, 1):
                ps = psum.tile([H, G4], f32, tag="gates")
                ps_tiles[(t, s)] = ps
                rhs = xsb[:, t * NPC + s * NS: t * NPC + (s + 1) * NS]
                for g in range(4):
                    # g==0 start=True zeroes the whole 2KB bank; later gates
                    # land on pending-zero bytes and overwrite.
                    nc.tensor.matmul(ps[:, g * NS:(g + 1) * NS],
                                     lhsT=wih_sb[:, g * H:(g + 1) * H],
                                     rhs=rhs, start=(g == 0), stop=True,
                                     skip_group_check=(g != 0))

        emit_x(0)
        h2sl = [None, None]
        for t in range(T):
            pss = [ps_tiles.pop((t, 0)), ps_tiles.pop((t, 1))]
            if t > 0:
                for s in (0, 1):
                    for g in range(4):
                        nc.tensor.matmul(pss[s][:, g * NS:(g + 1) * NS],
                                         lhsT=whh_sb[:, g * H:(g + 1) * H],
                                         rhs=h2sl[s], start=False, stop=True,
                                         skip_group_check=True)
            if t + 1 < T:
                emit_x(t + 1)
            Wc = [W[s][t % 2] for s in (0, 1)]
            Wn = [W[s][(t + 1) % 2] for s in (0, 1)]
            # scalar: both gate-tanh calls back-to-back so stream B's can
            # run while stream A's DVE chain executes
            for s in (0, 1):
                nc.scalar.activation(Wc[s][:, 0:4 * NS], pss[s][:, 0:4 * NS],
                                     AF.Tanh)
            # vector: uv_A, C2'_A, uv_B, C2'_B (keeps A's chain moving
            # before B's gate-tanh lands)
            for s in (0, 1):
                w = Wc[s]
                nc.vector.scalar_tensor_tensor(
                    out=pair_ap(w[:, 5 * NS:6 * NS], NS),      # [u | v]
                    in0=pair_ap(w[:, NS:2 * NS], NS),          # [t_i | t_f]
                    scalar=1.0,
                    in1=pair_ap(w[:, 0:NS], 4 * NS),           # [t_g | C2]
                    op0=ALU.add, op1=ALU.mult)
                nc.vector.scalar_tensor_tensor(
                    out=Wn[s][:, 4 * NS:5 * NS],               # C2' = 0.5v+u
                    in0=w[:, 6 * NS:7 * NS], scalar=0.5,
                    in1=w[:, 5 * NS:6 * NS],
                    op0=ALU.mult, op1=ALU.add)
            ys = []
            for s in (0, 1):
                y = work.tile([H, NS], f16, tag="y")
                nc.scalar.activation(y, Wn[s][:, 4 * NS:5 * NS], AF.Tanh,
                                     scale=0.5)
                ys.append(y)
            for s in (0, 1):
                rsl = rep[:, t * NPC + s * NS: t * NPC + (s + 1) * NS]
                nc.vector.scalar_tensor_tensor(
                    out=rsl, in0=Wc[s][:, 3 * NS:4 * NS], scalar=1.0,
                    in1=ys[s], op0=ALU.add, op1=ALU.mult)  # h2 = (t_o+1)*y
                h2sl[s] = rsl

        # ---- HA attention pooling over t (rep layout [h, (t, n)]).
        # Chunks of 8 sequences (free = 64t x 8n = 512), grouped by 4 for
        # the softmax/weighted-sum phase (32 seqs per group).
        rep3 = rep.rearrange("p (t n) -> p t n", n=NPC)
        NPCH = 8
        GRP = 4
        GN = GRP * NPCH  # 32 seqs per group
        ssum = state.tile([H, NPC], f32, tag="ssum")
        rr = state.tile([H, NPC], f32, tag="rr")
        stku = state.tile([H, NPC], f32, tag="stku")
        stock_sb = state.tile([H, NPC], f16, tag="stock_sb")
        hl0 = (T - 1) * NPC
        for grp in range(NPC // GN):
            wps = psumw.tile([H, GRP * T * NPCH], f32, tag="wps")
            for lc in range(GRP):
                ch = grp * GRP + lc
                ns = slice(ch * NPCH, (ch + 1) * NPCH)
                aps = psum.tile([H, G4], f32, tag="gates")
                aps3 = aps.rearrange("p (t n) -> p t n", n=NPCH)
                nc.tensor.matmul(aps3, lhsT=w1T_sb, rhs=rep3[:, :, ns],
                                 start=True, stop=False)
                # a2 contribution: h2_last broadcast over t via zero-stride AP
                hsl = rep[:, hl0 + ch * NPCH: hl0 + (ch + 1) * NPCH]
                hsl_b = bass.AP(tensor=hsl.tensor, offset=hsl.offset,
                                ap=[hsl.ap[0], [0, T], hsl.ap[1]])
                nc.tensor.matmul(aps3, lhsT=w2T_sb, rhs=hsl_b,
                                 start=False, stop=True)
                z = work.tile([H, G4], f16, tag="z")
                nc.scalar.activation(z, aps, AF.Tanh, bias=b12_sb)
                nc.tensor.matmul(wps[:, lc * G4:(lc + 1) * G4], lhsT=waT_sb,
                                 rhs=z, start=True, stop=True)
            nsl = slice(grp * GN, (grp + 1) * GN)
            GW = GRP * T * NPCH  # 2048
            # wps rows are replicated across all 128 partitions; softmax +
            # weighted sum run full-lane. wps col order is (lc, t, j); the
            # exp writes eU PERMUTED to (t, lc*8+j) so everything downstream
            # is (t, n)-ordered and matches rep (contiguous inner n).
            eU = work.tile([H, GW], f16, tag="eU")
            wps_v = bass.AP(tensor=wps.tensor, offset=wps.offset,
                            ap=[wps.ap[0], [T * NPCH, GRP], [NPCH, T],
                                [1, NPCH]])             # [p, lc, t, j]
            eU_p = bass.AP(tensor=eU.tensor, offset=eU.offset,
                           ap=[eU.ap[0], [NPCH, GRP], [GN, T],
                               [1, NPCH]])              # same logical dims
            nc.scalar.activation(eU_p, wps_v, AF.Exp)
            # sum over t: fold-tree of contiguous halves (t is outer in eU)
            red = work.tile([H, GW // 2], f16, tag="red")
            nc.vector.tensor_add(red, eU[:, 0:GW // 2], eU[:, GW // 2:GW])
            w_ = GW // 4
            while w_ >= 2 * GN:
                nc.vector.tensor_add(red[:, 0:w_], red[:, 0:w_],
                                     red[:, w_:2 * w_])
                w_ //= 2
            nc.vector.tensor_add(ssum[:, nsl], red[:, 0:GN],
                                 red[:, GN:2 * GN])
            nc.vector.reciprocal(rr[:, nsl], ssum[:, nsl])
            eU3 = eU.rearrange("p (t n) -> p t n", n=GN)
            nc.vector.tensor_mul(eU3, eU3, rep3[:, :, nsl])
            red2 = work.tile([H, GW // 2], f16, tag="red")
            nc.vector.tensor_add(red2, eU[:, 0:GW // 2], eU[:, GW // 2:GW])
            w_ = GW // 4
            while w_ >= 2 * GN:
                nc.vector.tensor_add(red2[:, 0:w_], red2[:, 0:w_],
                                     red2[:, w_:2 * w_])
                w_ //= 2
            nc.vector.tensor_add(stku[:, nsl], red2[:, 0:GN],
                                 red2[:, GN:2 * GN])
            # stock = 0.5 * stku * rr  (0.5 undoes the h2 = 2h scaling)
            nc.vector.scalar_tensor_tensor(
                out=stock_sb[:, nsl], in0=stku[:, nsl], scalar=0.5,
                in1=rr[:, nsl], op0=ALU.mult, op1=ALU.mult)

        # ---- cross-core gather of stock reps (cores 0-3 = batch 0,
        # cores 4-7 = batch 1), then CAAN. This core's own stock block IS
        # its query block (qc = core%4), so only K/V need the gather.
        stk_in = dram.tile([H, NPC], f16, tag="stk_in")
        xr_b = dram.tile([4 * H, NPC], f16, tag="xr_b")
        nc.sync.dma_start(out=stk_in, in_=stock_sb)
        nc.gpsimd.collective_compute(
            "AllGather", ALU.bypass,
            replica_groups=[[0, 1, 2, 3], [4, 5, 6, 7]],
            ins=[stk_in.opt()], outs=[xr_b.opt()])
        xrT_sb = state.tile([H, M], f16, tag="xrT")
        for r in range(4):
            nc.sync.dma_start(out=xrT_sb[:, r * NPC:(r + 1) * NPC],
                              in_=xr_b[r * H:(r + 1) * H, :])
        xqT_sb = stock_sb

        wqT_sb = wpk_sb[:, 0:H]
        wkT_sb = wpk_sb[:, H:2 * H]
        wvT_sb = wpk_sb[:, 2 * H:3 * H]
        eye_sb = wpk_sb[:, 3 * H:4 * H]
        wwT_sb = wpk_sb[:, 4 * H:4 * H + 1]
        bq_sb = bpk_sb[:, 0:1]
        bk_sb = bpk_sb[:, 1:2]
        cst_sb = bpk_sb[:, 2:3]

        # q/k projections (transposed layout [h', *])
        qps = psum.tile([H, NPC], f32, tag="gates")
        nc.tensor.matmul(qps, lhsT=wqT_sb, rhs=xqT_sb, start=True, stop=True)
        qsb = state.tile([H, NPC], f16, tag="qsb")
        nc.scalar.activation(qsb, qps, AF.Identity, bias=bq_sb)

        kps = psumw.tile([H, M], f32, tag="wps")
        nc.tensor.matmul(kps, lhsT=wkT_sb, rhs=xrT_sb, start=True, stop=True)
        ksb = state.tile([H, M], f16, tag="ksb")
        nc.scalar.activation(ksb, kps, AF.Identity, bias=bk_sb)

        # v in [k, h'] layout (no bias: beta rows sum to 1, folded into cst)
        vsb = state.tile([H, 4, H], f16, tag="vsb")
        for j in range(4):
            vps = psum.tile([H, H], f32, tag="gates")
            nc.tensor.matmul(vps, lhsT=xrT_sb[:, j * H:(j + 1) * H],
                             rhs=wvT_sb, start=True, stop=True)
            nc.scalar.activation(vsb[:, j, :], vps, AF.Copy)

        # S = q^T k / sqrt(H); e = exp
        sps = psum.tile([NPC, M], f32, tag="gates")
        nc.tensor.matmul(sps, lhsT=qsb, rhs=ksb, start=True, stop=True)
        esb = state.tile([NPC, M], f16, tag="esb")
        nc.scalar.activation(esb, sps, AF.Exp, scale=float(1.0 / np.sqrt(H)))
        essum = state.tile([NPC, 1], f32, tag="essum")
        nc.vector.tensor_reduce(essum, esb, mybir.AxisListType.X, ALU.add)
        err = state.tile([NPC, 1], f32, tag="err")
        nc.vector.reciprocal(err, essum)
        nc.vector.tensor_scalar_mul(esb, esb, err)

        # transpose e chunks -> eT [k, q], then attnT = sum_j v_j @ eT_j
        eT = state.tile([H, 4, NPC], f16, tag="eT")
        for j in range(4):
            tps = psum.tile([H, NPC], f16, tag="gates")
            nc.tensor.transpose(tps, esb[:, j * H:(j + 1) * H], eye_sb)
            nc.vector.tensor_copy(eT[:, j, :], tps)
        aps2 = psum.tile([H, NPC], f32, tag="gates")
        for j in range(4):
            nc.tensor.matmul(aps2, lhsT=vsb[:, j, :], rhs=eT[:, j, :],
                             start=(j == 0), stop=(j == 3))
        attn = state.tile([H, NPC], f16, tag="attn")
        nc.scalar.activation(attn, aps2, AF.Copy)

        scps = psum.tile([1, NPC], f32, tag="gates")
        nc.tensor.matmul(scps, lhsT=wwT_sb, rhs=attn, start=True, stop=True)
        ssb = state.tile([1, NPC], f32, tag="ssb")
        nc.scalar.activation(ssb, scps, AF.Identity, bias=cst_sb[0:1, :])
        nc.sync.dma_start(out=scores, in_=ssb)

    nc.compile()
    return nc


def _prep_inputs_a(inputs):
    perm = _gate_perm()
    # gate order (g, i, f, o); i/f/o pre-scaled by 1/2 (tanh-only gates),
    # W_hh additionally halved because the kernel feeds h2 = 2h.
    gscale = np.concatenate([np.ones(H, np.float32),
                             np.full(3 * H, 0.5, np.float32)])
    W_ih = np.asarray(inputs["W_ih"])[perm] * gscale[:, None]    # [512, 16]
    W_hh = np.asarray(inputs["W_hh"])[perm] * (0.5 * gscale)[:, None]
    bias = (np.asarray(inputs["b_ih"]) + np.asarray(inputs["b_hh"]))[perm]
    bias = bias * gscale
    wih = np.concatenate([W_ih.T, bias[None, :]], axis=0)  # [17, 512]
    whh = np.ascontiguousarray(W_hh.T)                     # [128, 512]
    # rep holds h2 = 2h -> halve w1/w2
    w1T = np.ascontiguousarray(np.asarray(inputs["w1"]).T) * 0.5
    w2T = np.ascontiguousarray(np.asarray(inputs["w2"]).T) * 0.5
    b12 = (np.asarray(inputs["b1"]) + np.asarray(inputs["b2"]))[:, None]
    waT = np.repeat(np.asarray(inputs["wa"]).T, H, axis=1)  # [128, 128] replicated

    x = np.asarray(inputs["x"]).reshape(B * M, T, F)
    wpk, bpk = _prep_caan(inputs)
    shared = dict(wih=np.ascontiguousarray(wih).astype(np.float16),
                  whh=whh.astype(np.float16),
                  w1T=w1T.astype(np.float16), w2T=w2T.astype(np.float16),
                  b12=np.ascontiguousarray(b12, np.float32),
                  waT=waT.astype(np.float16),
                  wpk=wpk, bpk=bpk)
    in_maps = []
    for c in range(N_CORES):
        xc = x[c * NPC:(c + 1) * NPC]                # [128, 64, 16]
        xTc = np.empty((F + 1, T * NPC), np.float16)
        xTc[:F] = xc.transpose(2, 1, 0).reshape(F, T * NPC)  # [f, t*128+n]
        xTc[F] = 1.0
        in_maps.append(dict(xT=np.ascontiguousarray(xTc), **shared))
    return in_maps


def _prep_caan(inputs):
    wqT = np.ascontiguousarray(np.asarray(inputs["wq"]).T).astype(np.float16)
    wkT = np.ascontiguousarray(np.asarray(inputs["wk"]).T).astype(np.float16)
    wvT = np.ascontiguousarray(np.asarray(inputs["wv"]).T).astype(np.float16)
    bq = np.ascontiguousarray(np.asarray(inputs["bq"])[:, None], np.float32)
    bk = np.ascontiguousarray(np.asarray(inputs["bk"])[:, None], np.float32)
    ww = np.asarray(inputs["ww"])                     # [1, H]
    bv = np.asarray(inputs["bv"])                     # [H]
    bw = np.asarray(inputs["bw"])                     # [1]
    wwT = np.ascontiguousarray(ww.T).astype(np.float16)
    cst = float(ww[0] @ bv + bw[0])
    eye = np.eye(H, dtype=np.float16)
    wpk = np.concatenate([wqT, wkT, wvT, eye, wwT], axis=1)
    bpk = np.concatenate([bq, bk, np.full((H, 1), cst, np.float32)], axis=1)
    return np.ascontiguousarray(wpk), np.ascontiguousarray(bpk)


def _get_programs():
    if "a" not in _CACHE:
        _CACHE["a"] = _build_launch_a()
    return _CACHE["a"]


def _assemble_scores(results):
    out = np.empty((B, M), np.float32)
    for c in range(N_CORES):
        b, qc = c // 4, c % 4
        out[b, qc * NPC:(qc + 1) * NPC] = results[c]["scores"][0]
    return out


def kernel(**inputs):
    from concourse.bass_utils import run_bass_kernel_spmd

    nca = _get_programs()
    in_maps = _prep_inputs_a(inputs)
    res = run_bass_kernel_spmd(nca, in_maps, core_ids=list(range(N_CORES)))
    return _assemble_scores(res.results)


# revision 14
# speedup vs baseline: 1.1312x; 1.1312x over previous
"""AlphaStock Trainium2 kernel (8 NeuronCores, SPMD).

Model: per-asset LSTM(T=64, H=128) + temporal attention pooling (HA), then
cross-asset attention (CAAN) over M=512 assets per batch element.

Sharding: the B*M=1024 sequences are split 128-per-core for the LSTM/HA
stage (launch A). The tiny CAAN stage runs as a second launch (B) sharded
by query rows (cores 0-3 -> batch 0, cores 4-7 -> batch 1), with the
gathered per-asset representations re-broadcast by the host between the
two launches.

Launch A inner loop (v2): the 128 local sequences run as TWO interleaved
streams of 64 so each stream's serial recurrence chain hides behind the
other stream's engine work. All four LSTM gates go through a SINGLE
tanh ACTIVATE per stream-step: the i/f/o rows of W_ih/W_hh are
pre-scaled by 1/2 on the host so sigmoid(x) = 0.5*tanh(x/2)+0.5 drops
out of the algebra. The cell state is kept as C2 = 2c and the hidden
output as h2 = 2h, which removes every standalone scale op:
    [t_g|t_i|t_f|t_o] = tanh(gates)               (1 ACT, PSUM src)
    [u|v] = (in+1)*[t_g|C2]  (paired STT)         (1 DVE op)
    C2'   = 0.5*v + u        (STT)                (1 DVE op)
    y     = tanh(0.5*C2')                         (1 ACT)
    h2    = (t_o+1)*y        (STT, written straight into rep)
rep is stored [h, (t, n)] so the h2 write is contiguous (no gpsimd
archive copy); w1/w2 are pre-halved and the final stock scale absorbs
the remaining factor of 2.
"""

import numpy as np

B, M, T, F, H = 2, 512, 64, 16, 128
N_CORES = 8
NPC = (B * M) // N_CORES  # sequences per core = 128
NS = NPC // 2             # sequences per stream = 64
G4 = 4 * H  # 512

_CACHE = {}


def _gate_perm():
    # torch gate order (i, f, g, o) -> kernel order (g, i, f, o)
    idx = np.arange(4 * H).reshape(4, H)
    return np.concatenate([idx[2], idx[0], idx[1], idx[3]])


def _build_launch_a():
    import concourse.bacc as bacc
    import concourse.tile as tile
    import concourse.bass as bass
    from concourse import mybir
    from contextlib import ExitStack

    f32 = mybir.dt.float32
    f16 = mybir.dt.float16
    AF = mybir.ActivationFunctionType
    ALU = mybir.AluOpType

    nc = bacc.Bacc("TRN2", target_bir_lowering=False, debug=False,
                   num_devices=N_CORES)

    xT = nc.dram_tensor("xT", [F + 1, T * NPC], f16, kind="ExternalInput").ap()
    wih = nc.dram_tensor("wih", [F + 1, G4], f16, kind="ExternalInput").ap()
    whh = nc.dram_tensor("whh", [H, G4], f16, kind="ExternalInput").ap()
    w1T = nc.dram_tensor("w1T", [H, H], f16, kind="ExternalInput").ap()
    w2T = nc.dram_tensor("w2T", [H, H], f16, kind="ExternalInput").ap()
    b12 = nc.dram_tensor("b12", [H, 1], f32, kind="ExternalInput").ap()
    waT = nc.dram_tensor("waT", [H, H], f16, kind="ExternalInput").ap()
    stock = nc.dram_tensor("stock", [H, NPC], f32, kind="ExternalOutput").ap()

    with tile.TileContext(nc) as tc, ExitStack() as ctx:
        big = ctx.enter_context(tc.tile_pool(name="big", bufs=1))
        state = ctx.enter_context(tc.tile_pool(name="state", bufs=1))
        work = ctx.enter_context(tc.tile_pool(name="work", bufs=3))
        psum = ctx.enter_context(tc.tile_pool(name="psum", bufs=4, space="PSUM"))
        psumw = ctx.enter_context(tc.tile_pool(name="psumw", bufs=1, space="PSUM"))

        # ---- resident tensors
        xsb = big.tile([F + 1, T * NPC], f16, tag="xsb")   # x[f, (t, n)]
        rep = big.tile([H, T * NPC], f16, tag="rep")       # h2[h, (t, n)]

        wih_sb = state.tile([F + 1, G4], f16, tag="wih")
        whh_sb = state.tile([H, G4], f16, tag="whh")
        w1T_sb = state.tile([H, H], f16, tag="w1T")
        w2T_sb = state.tile([H, H], f16, tag="w2T")
        b12_sb = state.tile([H, 1], f32, tag="b12")
        waT_sb = state.tile([H, H], f16, tag="waT")

        # x chunk 0 + LSTM weights first (they gate step 0); everything the
        # tail stages need rides the gpsimd queue in parallel.
        XCH = T * NPC // 8
        nc.sync.dma_start(out=wih_sb, in_=wih)
        nc.sync.dma_start(out=xsb[:, 0:XCH], in_=xT[:, 0:XCH])
        nc.sync.dma_start(out=whh_sb, in_=whh)
        for j in range(1, 8):
            nc.sync.dma_start(out=xsb[:, j * XCH:(j + 1) * XCH],
                              in_=xT[:, j * XCH:(j + 1) * XCH])
        nc.gpsimd.dma_start(out=w1T_sb, in_=w1T)
        nc.gpsimd.dma_start(out=w2T_sb, in_=w2T)
        nc.gpsimd.dma_start(out=b12_sb, in_=b12)
        nc.gpsimd.dma_start(out=waT_sb, in_=waT)

        # Per-stream double-buffered step tile:
        # cols [t_g | t_i | t_f | t_o | C2 | u | v], each NS wide.
        W = [[state.tile([H, 7 * NS], f16, tag=f"W{s}{p}", name=f"W{s}{p}")
              for p in (0, 1)] for s in (0, 1)]
        for s in (0, 1):
            nc.vector.memset(W[s][0][:, 4 * NS:5 * NS], 0.0)  # C2_0 = 0

        def pair_ap(base, stride):
            # [p, 2, NS] view: two NS-wide blocks `stride` cols apart
            return bass.AP(tensor=base.tensor, offset=base.offset,
                           ap=[base.ap[0], [stride, 2], [1, NS]])

        ps_tiles = {}

        def emit_x(t):
            for s in (0, 1):
                ps = psum.tile([H, G4], f32, tag="gates")
                ps_tiles[(t, s)] = ps
                rhs = xsb[:, t * NPC + s * NS: t * NPC + (s + 1) * NS]
                for g in range(4):
                    # g==0 start=True zeroes the whole 2KB bank; later gates
                    # land on pending-zero bytes and overwrite.
                    nc.tensor.matmul(ps[:, g * NS:(g + 1) * NS],
                                     lhsT=wih_sb[:, g * H:(g + 1) * H],
                                     rhs=rhs, start=(g == 0), stop=True,
                                     skip_group_check=(g != 0))

        emit_x(0)
        h2sl = [None, None]
        for t in range(T):
            pss = [ps_tiles.pop((t, 0)), ps_tiles.pop((t, 1))]
            if t > 0:
                for s in (0, 1):
                    for g in range(4):
                        nc.tensor.matmul(pss[s][:, g * NS:(g + 1) * NS],
                                         lhsT=whh_sb[:, g * H:(g + 1) * H],
                                         rhs=h2sl[s], start=False, stop=True,
                                         skip_group_check=True)
            if t + 1 < T:
                emit_x(t + 1)
            Wc = [W[s][t % 2] for s in (0, 1)]
            Wn = [W[s][(t + 1) % 2] for s in (0, 1)]
            # scalar: both gate-tanh calls back-to-back so stream B's can
            # run while stream A's DVE chain executes
            for s in (0, 1):
                nc.scalar.activation(Wc[s][:, 0:4 * NS], pss[s][:, 0:4 * NS],
                                     AF.Tanh)
            # vector: uv_A, C2'_A, uv_B, C2'_B (keeps A's chain moving
            # before B's gate-tanh lands)
            for s in (0, 1):
                w = Wc[s]
                nc.vector.scalar_tensor_tensor(
                    out=pair_ap(w[:, 5 * NS:6 * NS], NS),      # [u | v]
                    in0=pair_ap(w[:, NS:2 * NS], NS),          # [t_i | t_f]
                    scalar=1.0,
                    in1=pair_ap(w[:, 0:NS], 4 * NS),           # [t_g | C2]
                    op0=ALU.add, op1=ALU.mult)
                nc.vector.scalar_tensor_tensor(
                    out=Wn[s][:, 4 * NS:5 * NS],               # C2' = 0.5v+u
                    in0=w[:, 6 * NS:7 * NS], scalar=0.5,
                    in1=w[:, 5 * NS:6 * NS],
                    op0=ALU.mult, op1=ALU.add)
            ys = []
            for s in (0, 1):
                y = work.tile([H, NS], f16, tag="y")
                nc.scalar.activation(y, Wn[s][:, 4 * NS:5 * NS], AF.Tanh,
                                     scale=0.5)
                ys.append(y)
            for s in (0, 1):
                rsl = rep[:, t * NPC + s * NS: t * NPC + (s + 1) * NS]
                nc.vector.scalar_tensor_tensor(
                    out=rsl, in0=Wc[s][:, 3 * NS:4 * NS], scalar=1.0,
                    in1=ys[s], op0=ALU.add, op1=ALU.mult)  # h2 = (t_o+1)*y
                h2sl[s] = rsl

        # ---- HA attention pooling over t (rep layout [h, (t, n)]).
        # Chunks of 8 sequences (free = 64t x 8n = 512), grouped by 4 for
        # the softmax/weighted-sum phase (32 seqs per group).
        rep3 = rep.rearrange("p (t n) -> p t n", n=NPC)
        NPCH = 8
        GRP = 4
        GN = GRP * NPCH  # 32 seqs per group
        ssum = state.tile([H, NPC], f32, tag="ssum")
        rr = state.tile([H, NPC], f32, tag="rr")
        stku = state.tile([H, NPC], f32, tag="stku")
        stock_sb = state.tile([H, NPC], f32, tag="stock_sb")
        hl0 = (T - 1) * NPC
        for grp in range(NPC // GN):
            wps = psumw.tile([H, GRP * T * NPCH], f32, tag="wps")
            for lc in range(GRP):
                ch = grp * GRP + lc
                ns = slice(ch * NPCH, (ch + 1) * NPCH)
                aps = psum.tile([H, G4], f32, tag="gates")
                aps3 = aps.rearrange("p (t n) -> p t n", n=NPCH)
                nc.tensor.matmul(aps3, lhsT=w1T_sb, rhs=rep3[:, :, ns],
                                 start=True, stop=False)
                # a2 contribution: h2_last broadcast over t via zero-stride AP
                hsl = rep[:, hl0 + ch * NPCH: hl0 + (ch + 1) * NPCH]
                hsl_b = bass.AP(tensor=hsl.tensor, offset=hsl.offset,
                                ap=[hsl.ap[0], [0, T], hsl.ap[1]])
                nc.tensor.matmul(aps3, lhsT=w2T_sb, rhs=hsl_b,
                                 start=False, stop=True)
                z = work.tile([H, G4], f16, tag="z")
                nc.scalar.activation(z, aps, AF.Tanh, bias=b12_sb)
                nc.tensor.matmul(wps[:, lc * G4:(lc + 1) * G4], lhsT=waT_sb,
                                 rhs=z, start=True, stop=True)
            nsl = slice(grp * GN, (grp + 1) * GN)
            GW = GRP * T * NPCH  # 2048
            # wps rows are replicated across all 128 partitions; softmax +
            # weighted sum run full-lane. wps col order is (lc, t, j); the
            # exp writes eU PERMUTED to (t, lc*8+j) so everything downstream
            # is (t, n)-ordered and matches rep (contiguous inner n).
            eU = work.tile([H, GW], f16, tag="eU")
            wps_v = bass.AP(tensor=wps.tensor, offset=wps.offset,
                            ap=[wps.ap[0], [T * NPCH, GRP], [NPCH, T],
                                [1, NPCH]])             # [p, lc, t, j]
            eU_p = bass.AP(tensor=eU.tensor, offset=eU.offset,
                           ap=[eU.ap[0], [NPCH, GRP], [GN, T],
                               [1, NPCH]])              # same logical dims
            nc.scalar.activation(eU_p, wps_v, AF.Exp)
            # sum over t: fold-tree of contiguous halves (t is outer in eU)
            red = work.tile([H, GW // 2], f16, tag="red")
            nc.vector.tensor_add(red, eU[:, 0:GW // 2], eU[:, GW // 2:GW])
            w_ = GW // 4
            while w_ >= 2 * GN:
                nc.vector.tensor_add(red[:, 0:w_], red[:, 0:w_],
                                     red[:, w_:2 * w_])
                w_ //= 2
            nc.vector.tensor_add(ssum[:, nsl], red[:, 0:GN],
                                 red[:, GN:2 * GN])
            nc.vector.reciprocal(rr[:, nsl], ssum[:, nsl])
            eU3 = eU.rearrange("p (t n) -> p t n", n=GN)
            nc.vector.tensor_mul(eU3, eU3, rep3[:, :, nsl])
            red2 = work.tile([H, GW // 2], f16, tag="red")
            nc.vector.tensor_add(red2, eU[:, 0:GW // 2], eU[:, GW // 2:GW])
            w_ = GW // 4
            while w_ >= 2 * GN:
                nc.vector.tensor_add(red2[:, 0:w_], red2[:, 0:w_],
                                     red2[:, w_:2 * w_])
                w_ //= 2
            nc.vector.tensor_add(stku[:, nsl], red2[:, 0:GN],
                                 red2[:, GN:2 * GN])
            # stock = 0.5 * stku * rr  (0.5 undoes the h2 = 2h scaling)
            nc.vector.scalar_tensor_tensor(
                out=stock_sb[:, nsl], in0=stku[:, nsl], scalar=0.5,
                in1=rr[:, nsl], op0=ALU.mult, op1=ALU.mult)

        nc.sync.dma_start(out=stock, in_=stock_sb)

    nc.compile()
    return nc


def _prep_inputs_a(inputs):
    perm = _gate_perm()
    # gate order (g, i, f, o); i/f/o pre-scaled by 1/2 (tanh-only gates),
    # W_hh additionally halved because the kernel feeds h2 = 2h.
    gscale = np.concatenate([np.ones(H, np.float32),
                             np.full(3 * H, 0.5, np.float32)])
    W_ih = np.asarray(inputs["W_ih"])[perm] * gscale[:, None]    # [512, 16]
    W_hh = np.asarray(inputs["W_hh"])[perm] * (0.5 * gscale)[:, None]
    bias = (np.asarray(inputs["b_ih"]) + np.asarray(inputs["b_hh"]))[perm]
    bias = bias * gscale
    wih = np.concatenate([W_ih.T, bias[None, :]], axis=0)  # [17, 512]
    whh = np.ascontiguousarray(W_hh.T)                     # [128, 512]
    # rep holds h2 = 2h -> halve w1/w2
    w1T = np.ascontiguousarray(np.asarray(inputs["w1"]).T) * 0.5
    w2T = np.ascontiguousarray(np.asarray(inputs["w2"]).T) * 0.5
    b12 = (np.asarray(inputs["b1"]) + np.asarray(inputs["b2"]))[:, None]
    waT = np.repeat(np.asarray(inputs["wa"]).T, H, axis=1)  # [128, 128] replicated

    x = np.asarray(inputs["x"]).reshape(B * M, T, F)
    shared = dict(wih=np.ascontiguousarray(wih).astype(np.float16),
                  whh=whh.astype(np.float16),
                  w1T=w1T.astype(np.float16), w2T=w2T.astype(np.float16),
                  b12=np.ascontiguousarray(b12, np.float32),
                  waT=waT.astype(np.float16))
    in_maps = []
    for c in range(N_CORES):
        xc = x[c * NPC:(c + 1) * NPC]                # [128, 64, 16]
        xTc = np.empty((F + 1, T * NPC), np.float16)
        xTc[:F] = xc.transpose(2, 1, 0).reshape(F, T * NPC)  # [f, t*128+n]
        xTc[F] = 1.0
        in_maps.append(dict(xT=np.ascontiguousarray(xTc), **shared))
    return in_maps


def _build_launch_b():
    import concourse.bacc as bacc
    import concourse.tile as tile
    from concourse import mybir
    from contextlib import ExitStack

    f32 = mybir.dt.float32
    f16 = mybir.dt.float16
    AF = mybir.ActivationFunctionType

    nc = bacc.Bacc("TRN2", target_bir_lowering=False, debug=False,
                   num_devices=N_CORES)

    xrT = nc.dram_tensor("xrT", [H, M], f16, kind="ExternalInput").ap()
    xqT = nc.dram_tensor("xqT", [H, NPC], f16, kind="ExternalInput").ap()
    # packed: [wqT | wkT | wvT | eye | wwT-col]
    wpk = nc.dram_tensor("wpk", [H, 4 * H + 1], f16, kind="ExternalInput").ap()
    # packed: [bq | bk | cst-broadcast-row]
    bpk = nc.dram_tensor("bpk", [H, 3], f32, kind="ExternalInput").ap()
    scores = nc.dram_tensor("scores", [1, NPC], f32, kind="ExternalOutput").ap()

    with tile.TileContext(nc) as tc, ExitStack() as ctx:
        pool = ctx.enter_context(tc.tile_pool(name="sb", bufs=1))
        psum = ctx.enter_context(tc.tile_pool(name="ps", bufs=1, space="PSUM"))

        xrT_sb = pool.tile([H, M], f16, tag="xrT")
        xqT_sb = pool.tile([H, NPC], f16, tag="xqT")
        wpk_sb = pool.tile([H, 4 * H + 1], f16, tag="wpk")
        bpk_sb = pool.tile([H, 3], f32, tag="bpk")
        nc.sync.dma_start(out=xrT_sb, in_=xrT)
        nc.sync.dma_start(out=xqT_sb, in_=xqT)
        nc.sync.dma_start(out=wpk_sb, in_=wpk)
        nc.sync.dma_start(out=bpk_sb, in_=bpk)
        wqT_sb = wpk_sb[:, 0:H]
        wkT_sb = wpk_sb[:, H:2 * H]
        wvT_sb = wpk_sb[:, 2 * H:3 * H]
        eye_sb = wpk_sb[:, 3 * H:4 * H]
        wwT_sb = wpk_sb[:, 4 * H:4 * H + 1]
        bq_sb = bpk_sb[:, 0:1]
        bk_sb = bpk_sb[:, 1:2]
        cst_sb = bpk_sb[:, 2:3]

        # q/k projections (transposed layout [h', *])
        qps = psum.tile([H, NPC], f32, tag="ps")
        nc.tensor.matmul(qps, lhsT=wqT_sb, rhs=xqT_sb, start=True, stop=True)
        qsb = pool.tile([H, NPC], f16, tag="qsb")
        nc.scalar.activation(qsb, qps, AF.Identity, bias=bq_sb)

        kps = psum.tile([H, M], f32, tag="kps")
        nc.tensor.matmul(kps, lhsT=wkT_sb, rhs=xrT_sb, start=True, stop=True)
        ksb = pool.tile([H, M], f16, tag="ksb")
        nc.scalar.activation(ksb, kps, AF.Identity, bias=bk_sb)

        # v in [k, h'] layout (no bias: beta rows sum to 1, folded into cst)
        vsb = pool.tile([H, 4, H], f16, tag="vsb")
        for j in range(4):
            vps = psum.tile([H, H], f32, tag="ps")
            nc.tensor.matmul(vps, lhsT=xrT_sb[:, j * H:(j + 1) * H],
                             rhs=wvT_sb, start=True, stop=True)
            nc.scalar.activation(vsb[:, j, :], vps, AF.Copy)

        # S = q^T k / sqrt(H); e = exp
        sps = psum.tile([NPC, M], f32, tag="sps")
        nc.tensor.matmul(sps, lhsT=qsb, rhs=ksb, start=True, stop=True)
        esb = pool.tile([NPC, M], f16, tag="esb")
        nc.scalar.activation(esb, sps, AF.Exp, scale=float(1.0 / np.sqrt(H)))
        ssum = pool.tile([NPC, 1], f32, tag="ssum")
        nc.vector.tensor_reduce(ssum, esb, mybir.AxisListType.X,
                                mybir.AluOpType.add)
        rr = pool.tile([NPC, 1], f32, tag="rr")
        nc.vector.reciprocal(rr, ssum)
        nc.vector.tensor_scalar_mul(esb, esb, rr)

        # transpose e chunks -> eT [k, q], then attnT = sum_j v_j @ eT_j
        eT = pool.tile([H, 4, NPC], f16, tag="eT")
        for j in range(4):
            tps = psum.tile([H, NPC], f16, tag="tp")
            nc.tensor.transpose(tps, esb[:, j * H:(j + 1) * H], eye_sb)
            nc.vector.tensor_copy(eT[:, j, :], tps)
        aps = psum.tile([H, NPC], f32, tag="aps")
        for j in range(4):
            nc.tensor.matmul(aps, lhsT=vsb[:, j, :], rhs=eT[:, j, :],
                             start=(j == 0), stop=(j == 3))
        attn = pool.tile([H, NPC], f16, tag="attn")
        nc.scalar.activation(attn, aps, AF.Copy)

        scps = psum.tile([1, NPC], f32, tag="scps")
        nc.tensor.matmul(scps, lhsT=wwT_sb, rhs=attn, start=True, stop=True)
        ssb = pool.tile([1, NPC], f32, tag="ssb")
        nc.scalar.activation(ssb, scps, AF.Identity, bias=cst_sb[0:1, :])
        nc.sync.dma_start(out=scores, in_=ssb)

    nc.compile()
    return nc


def _prep_inputs_b(inputs, xr):
    # xr: [B, M, H] gathered stock_rep
    wqT = np.ascontiguousarray(np.asarray(inputs["wq"]).T).astype(np.float16)
    wkT = np.ascontiguousarray(np.asarray(inputs["wk"]).T).astype(np.float16)
    wvT = np.ascontiguousarray(np.asarray(inputs["wv"]).T).astype(np.float16)
    bq = np.ascontiguousarray(np.asarray(inputs["bq"])[:, None], np.float32)
    bk = np.ascontiguousarray(np.asarray(inputs["bk"])[:, None], np.float32)
    ww = np.asarray(inputs["ww"])                     # [1, H]
    bv = np.asarray(inputs["bv"])                     # [H]
    bw = np.asarray(inputs["bw"])                     # [1]
    wwT = np.ascontiguousarray(ww.T).astype(np.float16)
    cst = float(ww[0] @ bv + bw[0])
    eye = np.eye(H, dtype=np.float16)
    wpk = np.concatenate([wqT, wkT, wvT, eye, wwT], axis=1)
    bpk = np.concatenate([bq, bk, np.full((H, 1), cst, np.float32)], axis=1)
    wpk = np.ascontiguousarray(wpk)
    bpk = np.ascontiguousarray(bpk)

    in_maps = []
    for c in range(N_CORES):
        b, qc = c // 4, c % 4
        xrT = np.ascontiguousarray(xr[b].T).astype(np.float16)   # [H, M]
        xqT = np.ascontiguousarray(xrT[:, qc * NPC:(qc + 1) * NPC])
        in_maps.append(dict(xrT=xrT, xqT=xqT, wpk=wpk, bpk=bpk))
    return in_maps


def _gather_xr(results_a):
    xr = np.empty((B, M, H), np.float32)
    for c in range(N_CORES):
        st = results_a[c]["stock"]                   # [H, NPC]
        n0 = c * NPC
        b, m0 = divmod(n0, M)
        xr[b, m0:m0 + NPC] = st.T
    return xr


def _get_programs():
    if "a" not in _CACHE:
        _CACHE["a"] = _build_launch_a()
    if "b" not in _CACHE:
        _CACHE["b"] = _build_launch_b()
    return _CACHE["a"], _CACHE["b"]


def _assemble_scores(results_b):
    out = np.empty((B, M), np.float32)
    for c in range(N_CORES):
        b, qc = c // 4, c % 4
        out[b, qc * NPC:(qc + 1) * NPC] = results_b[c]["scores"][0]
    return out


def kernel(**inputs):
    from concourse.bass_utils import run_bass_kernel_spmd

    nca, ncb = _get_programs()
    in_maps_a = _prep_inputs_a(inputs)
    res_a = run_bass_kernel_spmd(nca, in_maps_a, core_ids=list(range(N_CORES)))
    xr = _gather_xr(res_a.results)
    in_maps_b = _prep_inputs_b(inputs, xr)
    res_b = run_bass_kernel_spmd(ncb, in_maps_b, core_ids=list(range(N_CORES)))
    return _assemble_scores(res_b.results)
